# revision 22
# baseline (speedup 1.0000x reference)
"""Distributed Trainium2 kernel for fused multi-head attention
(QKV projection + RoPE + softmax attention + output projection).

Problem: x[2,2048,1024], Wqkv[1024,3072], bqkv[3072], Wproj[1024,1024], bproj[1024]
NUM_HEADS=16, head_dim=64, non-causal, RoPE (half-split), scale hd^-0.5.

Sharding over 8 NeuronCores: 2-way batch x 4-way head-group tensor parallel.
Core c: batch b=c//4, head group g=c%4 (heads 4g..4g+3).

v3: scores matmuls are ROW-TILED -- two heads run concurrently in disjoint
64-row groups of the PE array (tile_position auto-derived from the base
partition), so the K=64 contraction needs no channel duplication and the
scores PE work halves. st tiles hold (headA | headB) halves of a 512-query
block in two adjacent PSUM banks; one activation covers both. exp is split
ScalarE:DVE 10:6 matching engine throughputs; ScalarE does only exp during
attention (normalize muls + strided reciprocals run on DVE, V-projection
evac on ScalarE pre-attention). One AllToAll per chunk; the first fires
mid-attention, the second's latency hides under chunk-0's output
projection. Startup: memset-based PE warmup (no DMA dependency), per-name
wq descriptors, x quart-0 descriptors first striped over 3 DMA queues.
"""
import sys

sys.path.insert(0, "/opt/trn_rl_repo")

import numpy as np
import ml_dtypes

BF16NP = ml_dtypes.bfloat16

N_CORES = 8
B, S, D = 2, 2048, 1024
H, HD = 16, 64
HPG = 4            # heads per group
NPAIR = 2          # head pairs per group (row-tiling unit)
TOK = S            # tokens per batch
KT = D // 128      # 8 contraction tiles for D
SK = S // 128      # 16 key tiles
QB = 4             # 512-query blocks
QBS = TOK // QB    # 512
ROWB = [0, 256]        # out_d row base per chunk
ROPE_BASE = 10000.0

TRACE = False
LAST_EXEC_NS = None
LAST_RES = None

# Schraudolph fast-exp in bf16 space: exp(x*0.125) ~= bitcast_bf16(
# int16(x * S_FE + B_FE)). scale 0.125 = hd^-0.5 (no duplication).
S_FE = 184.6645 * 0.125
B_FE = 16248.7

_CACHE = {}


def _build_nc():
    import concourse.bass as bass  # noqa
    import concourse.bacc as bacc
    import concourse.mybir as mybir
    from concourse import tile

    F32 = mybir.dt.float32
    BF16 = mybir.dt.bfloat16
    I16 = mybir.dt.int16
    AF = mybir.ActivationFunctionType
    ALU = mybir.AluOpType

    nc = bacc.Bacc("TRN2", target_bir_lowering=False, debug=False,
                   num_devices=N_CORES)

    # ---- per-core DRAM parameters, pre-arranged in SBUF layout on host ----
    xT_d = nc.dram_tensor("xT", [128, KT * TOK], BF16, kind="ExternalInput")
    wq_d = nc.dram_tensor("wq", [128, 4 * KT * 128], BF16, kind="ExternalInput")
    wv_d = nc.dram_tensor("wv", [128, KT * HPG * 65], BF16, kind="ExternalInput")
    wvo_d = nc.dram_tensor("wvo", [1, HPG * 65], BF16, kind="ExternalInput")
    cos_d = nc.dram_tensor("cosT", [128, TOK], BF16, kind="ExternalInput")
    sin_d = nc.dram_tensor("sinT", [128, TOK], BF16, kind="ExternalInput")
    bias_d = nc.dram_tensor("biases", [128, 4], F32, kind="ExternalInput")
    ones_b_d = nc.dram_tensor("ones_b", [1, 128], BF16, kind="ExternalInput")
    ident_d = nc.dram_tensor("ident", [128, 128], BF16, kind="ExternalInput")
    wp_d = nc.dram_tensor("wp", [128, KT * D], BF16, kind="ExternalInput")
    bp_d = nc.dram_tensor("bp", [1, D], BF16, kind="ExternalInput")
    out_d = nc.dram_tensor("out", [512, D], BF16, kind="ExternalOutput")

    with tile.TileContext(nc) as tc:
        with tc.tile_pool(name="const", bufs=1) as constp, \
             tc.tile_pool(name="persist", bufs=1) as persist, \
             tc.tile_pool(name="dram", bufs=1, space="DRAM") as dram:

            ones_b = constp.tile([1, 128], BF16)
            nc.sync.dma_start(ones_b[:], ones_b_d[:])
            ident = constp.tile([128, 128], BF16)  # loaded later (gpsimd)
            bias4 = constp.tile([128, 4], F32)
            nc.sync.dma_start(bias4[:], bias_d[:])
            bias_sb = {nm: bias4[:, i:i + 1]
                       for i, nm in enumerate(("qa", "qb", "ka", "kb"))}

            # RoPE'd q/k in PAIR layout: tile p holds heads 2p (rows 0:64)
            # and 2p+1 (rows 64:128); within a head, rows 0:32 = rotated
            # first-half channels (ar), rows 32:64 = second-half (br).
            qt = [persist.tile([128, TOK], BF16, name=f"qt{p}")
                  for p in range(NPAIR)]
            kt_ = [persist.tile([128, TOK], BF16, name=f"ktp{p}")
                   for p in range(NPAIR)]
            # V (ones-augmented): sk-block at cols sk*260, head h at +h*65
            vaug = persist.tile([128, SK * HPG * 65], BF16)
            # local attention output, per chunk: tok-tile t at cols t*256
            oloc = [persist.tile([128, 8 * 256], BF16, name=f"oloc{c}")
                    for c in range(2)]

            # A2A bounce buffers, one per chunk: [1024 tok, 256 = 4 heads]
            a2a_in = [dram.tile([1024, 256], BF16, name=f"a2a_in{c}")
                      for c in range(2)]
            a2a_out = [dram.tile([1024, 256], BF16, name=f"a2a_out{c}")
                       for c in range(2)]

            # ---------------- phase 1+2: QKV projection + RoPE ----------
            xv_ctx = tc.tile_pool(name="xv", bufs=1)
            xv = xv_ctx.__enter__()
            with tc.tile_pool(name="raw", bufs=1) as rawp:
                raw = {nm: rawp.tile([128, TOK], BF16, name=f"raw_{nm}")
                       for nm in ("qa", "qb", "ka", "kb")}

                with tc.tile_pool(name="xw", bufs=1) as xw, \
                     tc.tile_pool(name="qk_ps", bufs=4, space="PSUM") as qk_ps, \
                     tc.tile_pool(name="rope", bufs=2) as ropep:

                    # Startup DMA: spread descriptor issue over the 3 DMA-
                    # capable queues (sync/scalar HWDGE, gpsimd SWDGE), with
                    # everything the first QKV chunk needs issued first.
                    wall = xw.tile([128, 4 * KT * 128], BF16)
                    w_sb = {nm: wall[:, i * KT * 128:(i + 1) * KT * 128]
                            for i, nm in enumerate(("qa", "qb", "ka", "kb"))}
                    xt = [xv.tile([128, TOK], BF16, name=f"xt{k}")
                          for k in range(KT)]
                    engs = [nc.sync, nc.scalar, nc.gpsimd]

                    def wq_dma(eng, i):
                        eng.dma_start(wall[:, i * KT * 128:(i + 1) * KT * 128],
                                      wq_d[:, i * KT * 128:(i + 1) * KT * 128])

                    wq_dma(nc.sync, 0)       # qa
                    for k in range(KT):      # quart 0 of every k-tile
                        engs[k % 3].dma_start(
                            xt[k][:, 0:512], xT_d[:, k * TOK:k * TOK + 512])
                    wq_dma(nc.scalar, 1)     # qb
                    wq_dma(nc.sync, 2)       # ka
                    wq_dma(nc.scalar, 3)     # kb
                    for k in range(KT):      # quarts 1-3 in one descriptor
                        engs[k % 3].dma_start(
                            xt[k][:, 512:TOK],
                            xT_d[:, k * TOK + 512:k * TOK + TOK])

                    cosT = rawp.tile([128, TOK], BF16)
                    nc.gpsimd.dma_start(cosT[:], cos_d[:])
                    sinT = rawp.tile([128, TOK], BF16)
                    nc.gpsimd.dma_start(sinT[:], sin_d[:])
                    wv_sb = xv.tile([128, KT * HPG * 65], BF16)
                    nc.gpsimd.dma_start(wv_sb[:], wv_d[:])
                    wv_ones = xv.tile([1, HPG * 65], BF16)
                    nc.gpsimd.dma_start(wv_ones[:], wvo_d[:])

                    # PE warmup: dummy matmuls on a memset tile (no DMA
                    # dependency) keep the HAM clock gate warming during the
                    # startup DMA ramp
                    with tc.tile_pool(name="warm_ps", bufs=1,
                                      space="PSUM") as warm_ps, \
                         tc.tile_pool(name="warm_sb", bufs=1) as warm_sb:
                        wtile = warm_sb.tile([128, 128], BF16)
                        nc.vector.memset(wtile[:], 0.0)
                        wps = warm_ps.tile([128, 512], F32)
                        for i in range(52):
                            nc.tensor.matmul(wps[:, 0:128], wtile[:], wtile[:],
                                             start=True, stop=True)

                    # token-chunk loop: QKV matmuls + evac+bias
                    for ch in range(4):
                        c0, c1 = ch * 512, (ch + 1) * 512
                        for nm in ("qa", "qb", "ka", "kb"):
                            ps = qk_ps.tile([128, 512], F32, name="qkps",
                                            tag="qkps")
                            for k in range(KT):
                                nc.tensor.matmul(
                                    ps[:],
                                    w_sb[nm][:, k * 128:(k + 1) * 128],
                                    xt[k][:, c0:c1],
                                    start=(k == 0), stop=(k == KT - 1))
                            nc.scalar.add(
                                raw[nm][:, c0:c1], ps[:], bias_sb[nm][:])

                    # RoPE per 1024-half (stacked layout, full-lane DVE).
                    # Scatter into pair tiles: one 2-range DMA per (pair,
                    # src ar/br): ar rows 64p:64p+64 -> dst rows {0:32,64:96}
                    for half in range(2):
                        h0, h1 = half * 1024, (half + 1) * 1024
                        for pref in ("k", "q"):
                            a_r, b_r = raw[pref + "a"], raw[pref + "b"]
                            dst = qt if pref == "q" else kt_
                            m1 = ropep.tile([128, 1024], BF16, name="m1", tag="m1")
                            nc.vector.tensor_tensor(m1[:], a_r[:, h0:h1],
                                                    cosT[:, h0:h1], ALU.mult)
                            m2 = ropep.tile([128, 1024], BF16, name="m2", tag="m2")
                            nc.vector.tensor_tensor(m2[:], b_r[:, h0:h1],
                                                    sinT[:, h0:h1], ALU.mult)
                            ar = ropep.tile([128, 1024], BF16, name="ar", tag="ar")
                            nc.vector.tensor_tensor(ar[:], m1[:], m2[:],
                                                    ALU.subtract)
                            m3 = ropep.tile([128, 1024], BF16, name="m3", tag="m1")
                            nc.vector.tensor_tensor(m3[:], b_r[:, h0:h1],
                                                    cosT[:, h0:h1], ALU.mult)
                            m4 = ropep.tile([128, 1024], BF16, name="m4", tag="m2")
                            nc.vector.tensor_tensor(m4[:], a_r[:, h0:h1],
                                                    sinT[:, h0:h1], ALU.mult)
                            br = ropep.tile([128, 1024], BF16, name="br", tag="br")
                            nc.vector.tensor_tensor(br[:], m3[:], m4[:], ALU.add)
                            for p in range(NPAIR):
                                eng = nc.sync if pref == "k" else nc.gpsimd
                                for hh in range(2):
                                    r0 = 64 * p + 32 * hh
                                    eng.dma_start(
                                        dst[p][64 * hh:64 * hh + 32, h0:h1],
                                        ar[r0:r0 + 32, :])
                                    eng.dma_start(
                                        dst[p][64 * hh + 32:64 * hh + 64,
                                               h0:h1],
                                        br[r0:r0 + 32, :])

            # ---------------- phase 3: attention (sw-pipelined) ---------
            wpp_ctx = tc.tile_pool(name="wppool", bufs=1)
            wpp = wpp_ctx.__enter__()
            wp_sb = wpp.tile([128, KT * D], BF16)
            for quart in range(4):
                nc.gpsimd.dma_start(
                    wp_sb[:, quart * 2 * D:(quart + 1) * 2 * D],
                    wp_d[:, quart * 2 * D:(quart + 1) * 2 * D])
            bp_sb = wpp.tile([1, D], BF16)
            nc.gpsimd.dma_start(bp_sb[:], bp_d[:])
            nc.gpsimd.dma_start(ident[:], ident_d[:])

            # Rank-sync warm-up AllGather: absorbs PJRT dispatch skew while
            # the QKV phase runs so the real AllToAlls pay only wire time.
            sync_in = dram.tile([8, 16], BF16, name="sync_in")
            sync_out = dram.tile([64, 16], BF16, name="sync_out")
            nc.sync.dma_start(sync_in[:], ones_b_d[:].rearrange(
                "o (p n) -> (o p) n", p=8))
            nc.gpsimd.collective_compute(
                "AllGather", ALU.bypass,
                replica_groups=[[0, 1, 2, 3, 4, 5, 6, 7]],
                ins=[sync_in.opt()], outs=[sync_out.opt()])

            # block = (pair p, query-block qb). Order: finish both qb of a
            # chunk for one pair, ship + A2A that pair, move to next pair.
            BLOCKS = [(0, 0), (0, 1), (1, 0), (1, 1),
                      (0, 2), (0, 3), (1, 2), (1, 3)]
            est_map = {}

            with tc.tile_pool(name="st_ps", bufs=3, space="PSUM") as st_ps, \
                 tc.tile_pool(name="esb", bufs=48) as esb, \
                 tc.tile_pool(name="nrm", bufs=4) as nrmp:
                o_ps = None  # bound after the V-projection pool closes

                def emit_scores(blk):
                    p, qb = blk
                    base = qb * QBS
                    ktile, qtile = kt_[p], qt[p]
                    ests = []
                    for sk in range(SK):
                        # st halves: [head 2p | head 2p+1] for this 512-q
                        # block, in two adjacent PSUM banks; the two MMs are
                        # row-tiled (rows 0:64 / 64:128) and run concurrently
                        st = st_ps.tile([128, 1024], F32, name="st", tag="st")
                        for hh in range(2):
                            nc.tensor.matmul(
                                st[:, hh * 512:(hh + 1) * 512],
                                ktile[hh * 64:(hh + 1) * 64,
                                      sk * 128:(sk + 1) * 128],
                                qtile[hh * 64:(hh + 1) * 64,
                                      base:base + QBS],
                                start=True, stop=True)
                        # exp split: ScalarE true exp (~9.5/16) vs DVE
                        # Schraudolph fast-exp (~6.5/16) -- GpSimd cannot
                        # read PSUM. Ratio balances measured costs
                        # (ScalarE ~1.11us, DVE ~1.22us + norm work).
                        if sk in (1, 3, 5, 9, 11, 13) or \
                                (sk == 7 and qb % 2 == 0):
                            esti = esb.tile([128, 1024], I16,
                                            name=f"est{p}_{qb}_{sk}",
                                            tag="est")
                            nc.vector.tensor_scalar(
                                esti[:], st[:], S_FE, B_FE,
                                ALU.mult, ALU.add)
                            ests.append(esti[:].bitcast(BF16))
                        else:
                            est = esb.tile([128, 1024], BF16,
                                           name=f"est{p}_{qb}_{sk}",
                                           tag="est")
                            nc.scalar.activation(est[:], st[:], AF.Exp,
                                                 bias=0.0, scale=0.125)
                            ests.append(est[:])
                    est_map[blk] = ests

                def emit_pv(blk):
                    p, qb = blk
                    ci, qsub = qb // 2, (qb % 2) * 4
                    ests = est_map.pop(blk)
                    ops = [o_ps.tile([128, 260], F32,
                                     name=f"ops{p}_{qb}_{hh}", tag="ops")
                           for hh in range(2)]
                    for hh in range(2):
                        h = 2 * p + hh
                        for sub in range(4):
                            for sk in range(SK):
                                nc.tensor.matmul(
                                    ops[hh][:, sub * 65:sub * 65 + 65],
                                    ests[sk][:, hh * 512 + sub * 128:
                                             hh * 512 + (sub + 1) * 128],
                                    vaug[:, sk * (HPG * 65) + h * 65:
                                         sk * (HPG * 65) + h * 65 + 65],
                                    start=(sk == 0), stop=(sk == SK - 1))
                    # normalize on DVE: o / denom -> oloc (ScalarE stays
                    # dedicated to exp). One strided reciprocal per head.
                    for hh in range(2):
                        h = 2 * p + hh
                        po = ops[hh]
                        rec4 = nrmp.tile([128, 4], F32, name="rec", tag="rec")
                        nc.vector.reciprocal(
                            rec4[:], po[:].rearrange(
                                "q (s c) -> q s c", s=4)[:, :, 64])
                        for sub in range(4):
                            gs = qsub + sub
                            nc.vector.tensor_scalar(
                                oloc[ci][:, gs * 256 + h * 64:
                                         gs * 256 + h * 64 + 64],
                                po[:, sub * 65:sub * 65 + 64],
                                rec4[:, sub:sub + 1], None, ALU.mult)
                    # ship this pair's slice of the qb to the A2A bounce
                    nc.sync.dma_start(
                        a2a_in[ci][(qb % 2) * 512:(qb % 2) * 512 + 512,
                                   2 * p * 64:2 * p * 64 + 128].rearrange(
                            "(t q) n -> q t n", q=128),
                        oloc[ci][:].rearrange(
                            "q (t n) -> q t n", t=8)[:, qsub:qsub + 4,
                                                     2 * p * 64:
                                                     2 * p * 64 + 128])
                    if blk in ((1, 1), (1, 3)):
                        # A2A for chunk ci: 8 shards of 128 tokens x 256 ch
                        nc.gpsimd.collective_compute(
                            "AllToAll", ALU.bypass,
                            replica_groups=[[0, 1, 2, 3, 4, 5, 6, 7]],
                            ins=[a2a_in[ci].opt()],
                            outs=[a2a_out[ci].opt()])

                # V projection runs with v_ps(2) sharing PSUM with the st
                # ring (2 + 6 = 8 banks); block 0/1 scores + their exps
                # overlap the V tail and the rank-sync below.
                vps_ctx = tc.tile_pool(name="v_ps", bufs=2, space="PSUM")
                v_ps = vps_ctx.__enter__()
                for sk in range(SK):
                    ps = v_ps.tile([128, HPG * 65], F32, name="vps",
                                   tag="vps")
                    for k in range(KT):
                        nc.tensor.matmul(
                            ps[:],
                            xt[k][:, sk * 128:(sk + 1) * 128],
                            wv_sb[:, k * (HPG * 65):(k + 1) * (HPG * 65)],
                            start=(k == 0), stop=False)
                    nc.tensor.matmul(ps[:], ones_b[:], wv_ones[:],
                                     start=False, stop=True)
                    nc.scalar.add(
                        vaug[:, sk * (HPG * 65):(sk + 1) * (HPG * 65)],
                        ps[:], 0.0)

                emit_scores(BLOCKS[0])
                emit_scores(BLOCKS[1])

                # Blocking rank-sync: AllGather 1.0-bytes sourced from the
                # FIRST V denominator column (ready right after evac 0, so
                # the collective flies while V + block-0/1 scores run),
                # written back over two vaug denominator columns (1.0 over
                # 1.0 -- numerically exact). Every core's PV phase then
                # starts aligned, which shrinks the AllToAll walls (mesh
                # rounds amplify trigger skew).
                sync2_in = dram.tile([8, 1], BF16, name="sync2_in")
                sync2_out = dram.tile([64, 1], BF16, name="sync2_out")
                nc.sync.dma_start(sync2_in[:], vaug[0:8, 64:65])
                nc.gpsimd.collective_compute(
                    "AllGather", ALU.bypass,
                    replica_groups=[[0, 1, 2, 3, 4, 5, 6, 7]],
                    ins=[sync2_in.opt()], outs=[sync2_out.opt()])
                nc.sync.dma_start(vaug[0:64, 64:65], sync2_out[0:64, :])
                nc.sync.dma_start(vaug[0:64, 194:195], sync2_out[0:64, :])

                vps_ctx.__exit__(None, None, None)
                ops_ctx = tc.tile_pool(name="o_ps", bufs=2, space="PSUM")
                o_ps = ops_ctx.__enter__()
                for i, blk in enumerate(BLOCKS):
                    if i + 2 < len(BLOCKS):
                        emit_scores(BLOCKS[i + 2])
                    emit_pv(blk)
                ops_ctx.__exit__(None, None, None)

            # ---------------- phase 4: output projection ----------------
            shard = 128
            with tc.tile_pool(name="ot", bufs=16) as otp, \
                 tc.tile_pool(name="otin", bufs=8) as otinp, \
                 tc.tile_pool(name="tr_ps", bufs=3, space="PSUM") as tr_ps, \
                 tc.tile_pool(name="op_ps", bufs=3, space="PSUM") as op_ps, \
                 tc.tile_pool(name="osb", bufs=4) as osb:
                for ci in range(2):
                    # a2a_out[ci][cc] rows: sender r = (batch r//4, grp r%4);
                    # its 128-col block = heads {4g+2cc, 4g+2cc+1} -> o chan
                    # block k=2g+cc. Build oT via PE transposes.
                    for beta in range(2):
                        oin2 = []
                        for cc in range(2):
                            t = otinp.tile([shard, 4 * 128], BF16,
                                           name=f"oin{ci}_{beta}_{cc}",
                                           tag="oin")
                            nc.sync.dma_start(
                                t[:].rearrange("q (r n) -> q r n", r=4),
                                a2a_out[ci][512 * beta:512 * (beta + 1),
                                            cc * 128:(cc + 1) * 128]
                                .rearrange("(r q) n -> q r n", q=shard))
                            oin2.append(t)
                        ot = []
                        for k in range(KT):
                            g, cc = divmod(k, 2)
                            tp = tr_ps.tile([128, shard], F32, name="tp",
                                            tag="tp")
                            nc.tensor.matmul(
                                tp[:], oin2[cc][:, g * 128:(g + 1) * 128],
                                ident[0:shard, 0:shard],
                                start=True, stop=True)
                            o_t = otp.tile([128, shard], BF16,
                                           name=f"ot{ci}_{beta}_{k}", tag="ot")
                            # evac on ScalarE -- DVE still drains the last
                            # attention norm work when outproj starts
                            nc.scalar.add(o_t[:], tp[:], 0.0)
                            ot.append(o_t)
                        for ncol in range(2):
                            ps = op_ps.tile([shard, 512], F32, name="oppsum",
                                            tag="oppsum")
                            for k in range(KT):
                                nc.tensor.matmul(
                                    ps[:],
                                    ot[k][:],
                                    wp_sb[:, k * D + ncol * 512:
                                          k * D + (ncol + 1) * 512],
                                    start=(k == 0), stop=False)
                            nc.tensor.matmul(
                                ps[:], ones_b[0:1, 0:shard],
                                bp_sb[:, ncol * 512:(ncol + 1) * 512],
                                start=False, stop=True)
                            ob = osb.tile([shard, 512], BF16, name="ob",
                                          tag="ob")
                            nc.vector.tensor_copy(ob[:], ps[:])
                            eng = nc.sync if ncol == 0 else nc.scalar
                            eng.dma_start(
                                out_d[ROWB[ci] + beta * shard:
                                      ROWB[ci] + (beta + 1) * shard,
                                      ncol * 512:(ncol + 1) * 512],
                                ob[:])
            wpp_ctx.__exit__(None, None, None)
            xv_ctx.__exit__(None, None, None)
    nc.compile()
    return nc


def _prepare_inputs(x, Wqkv, bqkv, Wproj, bproj):
    """Build the 8 per-core input maps (host-side sharding only)."""
    W3 = Wqkv.reshape(D, 3, H, HD)
    b3 = bqkv.reshape(3, H, HD)

    def to_sbuf_layout(w):  # [D, N] -> [128, KT*N]
        n = w.shape[1]
        return np.ascontiguousarray(
            w.reshape(KT, 128, n).transpose(1, 0, 2).reshape(128, KT * n))

    # RoPE tables, stacked layout [128, TOK]: row j*32+c -> cos(ang[pos, c])
    inv = (1.0 / (ROPE_BASE ** (np.arange(0, HD, 2, dtype=np.float64) / HD)))
    ang = np.arange(TOK, dtype=np.float64)[:, None] * inv[None, :]  # [TOK, 32]
    cosT = np.tile(np.cos(ang).T.astype(np.float32), (4, 1)).astype(BF16NP)
    sinT = np.tile(np.sin(ang).T.astype(np.float32), (4, 1)).astype(BF16NP)

    wp_bf = to_sbuf_layout(Wproj).astype(BF16NP)
    bp_eff = (bqkv[2 * D:3 * D].astype(np.float64) @ Wproj.astype(np.float64)
              + bproj.astype(np.float64)).astype(np.float32)
    bp_bf = bp_eff[None, :].astype(BF16NP)
    ones_b = np.ones((1, 128), BF16NP)
    ident = np.eye(128, dtype=np.float32).astype(BF16NP)

    in_maps = []
    for c in range(N_CORES):
        b, g = divmod(c, 4)
        hs = slice(4 * g, 4 * g + 4)
        xT = to_sbuf_layout(
            np.ascontiguousarray(x[b].T)).astype(BF16NP)  # [128, KT*TOK]

        wq_parts = [
            W3[:, 0, hs, 0:32].reshape(D, 128),
            W3[:, 0, hs, 32:64].reshape(D, 128),
            W3[:, 1, hs, 0:32].reshape(D, 128),
            W3[:, 1, hs, 32:64].reshape(D, 128),
        ]
        wq = np.concatenate(
            [to_sbuf_layout(np.ascontiguousarray(w)) for w in wq_parts],
            axis=1).astype(BF16NP)  # [128, 4*KT*128]

        wv = np.zeros((D, HPG * 65), np.float32)
        wv.reshape(D, HPG, 65)[:, :, 0:64] = W3[:, 2, hs, :]
        wv = to_sbuf_layout(wv).astype(BF16NP)
        wvo = np.zeros((1, HPG * 65), np.float32)
        for j in range(HPG):
            wvo[0, j * 65 + 64] = 1.0
        wvo = wvo.astype(BF16NP)

        biases = np.stack([
            b3[0, hs, 0:32].reshape(128),
            b3[0, hs, 32:64].reshape(128),
            b3[1, hs, 0:32].reshape(128),
            b3[1, hs, 32:64].reshape(128),
        ], axis=1).astype(np.float32)  # [128, 4]

        in_maps.append({
            "xT": xT, "wq": wq, "wv": wv, "wvo": wvo,
            "cosT": cosT, "sinT": sinT, "biases": biases,
            "ones_b": ones_b, "ident": ident,
            "wp": wp_bf, "bp": bp_bf,
        })
    return in_maps


def kernel(x, Wqkv, bqkv, Wproj, bproj):
    global LAST_EXEC_NS, LAST_RES
    from concourse.bass_utils import run_bass_kernel_spmd

    if "nc" not in _CACHE:
        _CACHE["nc"] = _build_nc()
    nc = _CACHE["nc"]

    in_maps = _prepare_inputs(
        np.asarray(x, np.float32), np.asarray(Wqkv, np.float32),
        np.asarray(bqkv, np.float32), np.asarray(Wproj, np.float32),
        np.asarray(bproj, np.float32))

    kw = {}
    if TRACE:
        kw["trace"] = True
    res = run_bass_kernel_spmd(nc, in_maps, core_ids=list(range(N_CORES)), **kw)
    LAST_EXEC_NS = res.exec_time_ns
    LAST_RES = res

    out = np.empty((B, S, D), np.float32)
    for c in range(N_CORES):
        r = res.results[c]["out"].astype(np.float32)
        for ci in range(2):
            base = ci * 1024
            for beta in range(B):
                out[beta, base + c * 128:base + (c + 1) * 128] = \
                    r[ROWB[ci] + beta * 128:ROWB[ci] + (beta + 1) * 128]
    return out
